# revision 66
# baseline (speedup 1.0000x reference)
"""Trainium2 Bass kernel for nn_MultiHeadTransformerPosEmb.

Output is `y[:, 0, :] @ wu.T` — only the CLS row feeds the unembedding, so per
batch only ONE attention query row is needed; the [B,H,S,S] score tensor never
materializes.

v2 dataflow (per core = per batch):
 - pos-major embedding gather (transpose=False) -> Epos[pos, h] tiles feed the
   attention-weighted reduction U^T[h,n] = sum_s y[s,h] A[s,n] DIRECTLY as
   f=8 matmuls accumulating in PSUM (no [S,256] v tensor, no PSUM->SBUF v
   copies, no yT adds, no pe upconvert).
 - scores need hidden-major emb: 2 PE transposes per tile + batched
   PSUM->SBUF copies (bf16 2x DVE path).
 - pe rotation trick: pe[128k+t] = pe[t] x blockrot(k), so pe-scores for all
   16 tiles come from ONE 32KB pe0 table with per-tile rotated qk' (C/S
   tables), and U_pe comes from P'[h',n,k] = pe0^T@aT_k partials mixed with
   the same C/S tables.  No full [S,256] pe tensors in HBM at all.
 - 3-round XOR-hypercube all-gather of z (as baseline), vocab-parallel
   unembed, output via prepared dma_scatter_add (descgen hoisted to Pool idle
   time; firing skips the 625ns HWDGE + 650ns DGE delay of a dma_start).
 - DMA order hand-packed around the gather's descgen-gated window so the wu
   (unembedding weight) stream finishes right as z exits the exchange.

Precision plan (rel-err budget 2e-2): output dominated by the residual
2*y0 = 2*(emb[2] + pe[0]).  Hidden dim host-permuted to [odd, even] so
pe[0] = [1...1, 0...0]; wu odd half stays bf16, even half fp8.  Attention
weights/tables fp8, emb gather bf16.  Host-side prep is limited to
index/layout/dtype transforms.
"""
import math
from contextlib import ExitStack

import numpy as np
import ml_dtypes

import concourse.bass as bass
import concourse.tile as tile
import concourse.mybir as mybir
from concourse import bacc, bass_utils
from concourse.tile_rust import add_dep_helper

F32 = mybir.dt.float32
BF16 = mybir.dt.bfloat16
FP8 = mybir.dt.float8e4
I16 = mybir.dt.int16

NCORES = 8
B = 8
SEQ = 2048          # S + 1 (CLS prepended)
H = 256             # hidden
NH = 8              # heads
HD = 32             # head dim
V = 32001
VPAD = 32768
VSLICE = VPAD // NCORES   # 4096
NCH = VSLICE // 128       # 32 vocab chunks per core
NT = SEQ // 128           # 16 position tiles
NG = 4                    # score/exp groups (4 tiles each)
HSEQ = SEQ // 2
SCALE = 1.0 / math.sqrt(HD)

# hidden-dim permutation: odd dims first (pe row0 == 1), even dims second
# (pe row0 == 0).  Applied on the host to every tensor's hidden axis.
PERM = np.concatenate([np.arange(1, H, 2), np.arange(0, H, 2)])

# fp8 pack layout (columns of pack8 [128, 2, P8])
P_WQ = 0
P_WK = 256
P_PE0 = 512         # pe0 hidden-major [p=h-in-chunk, c, pos 0..127]
P_ID = 640          # identity 128x128 bf16, stored as byte pairs (c=0 plane)
P_MC = 896          # maskc [.., 8]
P_NM = 904          # nmask [row=(g,n,kk), 8]
P_SPLIT = 912       # ---- phase split: everything above loads pre-gather ----
P_WV = 912
P_WO = 1168
P8 = 1424

_CACHE = {}


def _build(attach_exchange_wait=True):
    nc = bacc.Bacc("TRN2", target_bir_lowering=False, debug=False,
                   num_devices=NCORES, num_swdge_queues=3)

    emb = nc.dram_tensor("emb", [V, H], BF16, kind="ExternalInput")
    idxs = nc.dram_tensor("idxs", [128, 144], I16, kind="ExternalInput")
    pack8 = nc.dram_tensor("pack8", [128, 2, P8], FP8, kind="ExternalInput")
    pep = nc.dram_tensor("pep", [128, NT, H], FP8, kind="ExternalInput")
    # ct/st rotation tables: [p, (ct|st), c, k, n]
    ctst = nc.dram_tensor("ctst", [128, 2, 2, NT, NH], FP8,
                          kind="ExternalInput")
    packm = nc.dram_tensor("packm", [128, 3], F32, kind="ExternalInput")
    maskt = nc.dram_tensor("maskt", [NH, H], F32, kind="ExternalInput")
    wu_hi = nc.dram_tensor("wu_hi", [128, VSLICE], BF16, kind="ExternalInput")
    wu_lo = nc.dram_tensor("wu_lo", [128, VSLICE], FP8, kind="ExternalInput")
    # out rows: r = half*128 + p (scatter idx); cols = ch*8 + b
    out = nc.dram_tensor("out", [256, 128], BF16, kind="ExternalOutput")
    rankmap = nc.dram_tensor("rankmap", [1, B], BF16, kind="ExternalOutput")

    with tile.TileContext(nc) as tc, ExitStack() as ctx:
        cp = ctx.enter_context(tc.tile_pool(name="const", bufs=1))
        wp = ctx.enter_context(tc.tile_pool(name="work", bufs=2))
        bigp = ctx.enter_context(tc.tile_pool(name="big", bufs=1))
        rxp = ctx.enter_context(tc.tile_pool(name="rx", bufs=1))
        # PSUM: 8 banks = tt2 + sp1 + pp1 + op1 + qp2 + up1
        ttp = ctx.enter_context(tc.tile_pool(name="tt", bufs=2, space="PSUM"))
        sp = ctx.enter_context(tc.tile_pool(name="sps", bufs=1, space="PSUM"))
        op = ctx.enter_context(tc.tile_pool(name="ops", bufs=1, space="PSUM"))
        qp = ctx.enter_context(tc.tile_pool(name="qps", bufs=2, space="PSUM"))
        up = ctx.enter_context(tc.tile_pool(name="ups", bufs=2, space="PSUM"))

        # ---- DMA issue order (SP queue; single serial DMA-engine device) ----
        # Pre-gather set: transfers must drain before the first gather's
        # descriptors reach the (arrival-ordered) DMA-engine queue at ~5.1us.
        _chain_last = {}

        def chain(key, instr):
            prev = _chain_last.get(key)
            if prev is not None:
                add_dep_helper(instr.ins, prev.ins, sync=False,
                               reason=f"{key} stream order")
            _chain_last[key] = instr
            return instr

        idx_sb = cp.tile([128, 144], I16)
        chain("sp", nc.sync.dma_start(idx_sb[:], idxs[:]))
        pep_sb = cp.tile([128, NT, H], FP8)
        chain("sp", nc.sync.dma_start(pep_sb[:], pep[:]))
        p8 = cp.tile([128, 2, P8], FP8)
        chain("sp", nc.sync.dma_start(p8[:, :, 0:P_SPLIT],
                                      pack8[:, :, 0:P_SPLIT]))
        cs = cp.tile([128, 2, 2, NT, NH], FP8)
        chain("sp", nc.sync.dma_start(cs[:], ctst[:]))
        pm = cp.tile([128, 3], F32)
        chain("sp", nc.sync.dma_start(pm[:], packm[:]))
        # (order: idx, pep, weights, tables, packm -- all transfers drain
        #  before the first gather's descriptors arrive at ~5.1us)
        wu_lo_sb = cp.tile([128, VSLICE], FP8)
        wu_hi_sb = cp.tile([128, VSLICE], BF16)
        mt = cp.tile([NH, H], F32)

        # pack views
        wq_sb = p8[:, :, P_WQ:P_WQ + H]
        wk_sb = p8[:, :, P_WK:P_WK + H]
        wv_sb = p8[:, :, P_WV:P_WV + H]
        wo_sb = p8[:, :, P_WO:P_WO + H]
        maskc_sb = p8[:, :, P_MC:P_MC + NH]
        nmask_sb = p8[0:32, 0, P_NM:P_NM + NH]
        ident_sb = p8[:, 0, P_ID:P_ID + 256].bitcast(BF16)
        pe0h = p8[:, :, P_PE0:P_PE0 + 128]              # [p=h, c, pos]
        ct4 = cs[:, 0]                                  # [128, 2, NH, NT]
        st4 = cs[:, 1]

        # ---- embedding gather (pos-major, bf16) -> Epos ---------------------
        ep = bigp.tile([128, NT, H], BF16, tag="epos", name="epos")
        gather_i = []
        for h in range(2):
            gather_i.append(nc.gpsimd.dma_gather(
                out_ap=ep[:, bass.ts(h, 8), :], in_ap=emb[:],
                idxs_ap=idx_sb[:, bass.ts(h, HSEQ // 16)],
                num_idxs=HSEQ, num_idxs_reg=HSEQ, elem_size=H, transpose=False,
                single_packet=False,
            ))

        # ---- post-gather DMAs -----------------------------------------------
        # Gate on a Pool no-op right after the 2nd gather's desc-gen: their
        # HWDGE+DGE latency then lands them in the DMA queue AFTER gather1's
        # descriptors (arrival order) but with zero idle gap once it drains.
        pxt = cp.tile([1, 2], I16)
        proxy = nc.gpsimd.memset(pxt[:], 0)
        add_dep_helper(proxy.ins, gather_i[1].ins, sync=False,
                       reason="proxy marks gather desc-gen done")
        wu_dmas = [
            chain("sp", nc.sync.dma_start(wu_lo_sb[:, 0:2048],
                                          wu_lo[:, 0:2048])),
            chain("sp", nc.sync.dma_start(wu_hi_sb[:, 0:1024],
                                          wu_hi[:, 0:1024])),
            chain("sp", nc.sync.dma_start(wu_hi_sb[:, 1024:2048],
                                          wu_hi[:, 1024:2048])),
            chain("sp", nc.sync.dma_start(p8[:, :, P_SPLIT:],
                                          pack8[:, :, P_SPLIT:])),
            chain("sp", nc.sync.dma_start(mt[:], maskt[:])),
            chain("sp", nc.sync.dma_start(wu_lo_sb[:, 2048:4096],
                                          wu_lo[:, 2048:4096])),
            chain("sp", nc.sync.dma_start(wu_hi_sb[:, 2048:3072],
                                          wu_hi[:, 2048:3072])),
            chain("sp", nc.sync.dma_start(wu_hi_sb[:, 3072:4096],
                                          wu_hi[:, 3072:4096])),
        ]
        for wdma in wu_dmas:
            add_dep_helper(wdma.ins, proxy.ins,
                           reason="post DMAs queue behind gather descriptors")

        # ---- exchange descriptor-gen, hoisted (XOR hypercube, 3 rounds) -----
        # zcol cols: [z_half0, z_half1, rank, pad].  zt slot k will hold the
        # payload of core (me XOR k); host un-permutes rows via the rank col.
        zt = rxp.tile([128, B, 4], BF16)
        zcol = zt[:, 0, :]
        # zcol bookkeeping cols, written BEFORE the rdma desc-gens are emitted
        # so the write-after-prep-read hazard points the right way
        zw = [nc.vector.memset(zcol[:, 2:4], 0.0),
              nc.vector.tensor_copy(out=zcol[0:1, 2:3], in_=pm[0:1, 2:3])]
        # Timing-only: remote writes land in a dummy so the no-exec timeline
        # sim (which never fires rdma DMA sems) doesn't inherit dead waits.
        # Identical shapes/costs; the real build uses the true addresses.
        ztd = zt if attach_exchange_wait else rxp.tile([128, B, 4], BF16,
                                                       tag="ztdummy")
        rsem1 = nc.alloc_semaphore("rx1")
        rsem2 = nc.alloc_semaphore("rx2")
        rsem3 = nc.alloc_semaphore("rx3")
        lsem = nc.alloc_semaphore("rdma_local")
        rd1 = [None] * NCORES
        rd1[1] = (0, 1)
        d1 = nc.gpsimd.remote_dma_broadcast(
            out_ap=ztd[:, 1, :], in_ap=zcol[:],
            remote_sem=rsem1, local_sem=lsem, rdests=rd1)
        rd2 = [None] * NCORES
        rd2[2] = (0, 2)
        d2 = nc.gpsimd.remote_dma_broadcast(
            out_ap=ztd[:, 2:4, :], in_ap=zt[:, 0:2, :],
            remote_sem=rsem2, local_sem=lsem, rdests=rd2)
        rd3 = [None] * NCORES
        rd3[4] = (0, 4)
        d3 = nc.gpsimd.remote_dma_broadcast(
            out_ap=ztd[:, 4:8, :], in_ap=zt[:, 0:4, :],
            remote_sem=rsem3, local_sem=lsem, rdests=rd3)
        for d in (d1, d2, d3):
            add_dep_helper(d.ins, gather_i[1].ins, sync=False,
                           reason="rdma desc-gen after gather desc dispatch")

        # ---- output staging buffer ------------------------------------------
        osb = bigp.tile([128, 2, 128], BF16, tag="osb", name="osb")

        ones_sb = cp.tile([128, 1], BF16)
        chain("dve", nc.vector.memset(ones_sb[:], 1.0))

        # ---- y0 = emb[2] + pe[0]; pe[0] is [1...,0...] after the perm -------
        y0f = wp.tile([128, 2, 1], F32, tag="y0f")
        chain("dve", nc.vector.tensor_scalar_add(y0f[:, 0, :], pm[:, 0:1], 1.0))
        chain("dve", nc.vector.tensor_copy(out=y0f[:, 1, :], in_=pm[:, 1:2]))
        y0b = wp.tile([128, 2, 1], BF16, tag="y0b")
        chain("dve", nc.vector.tensor_copy(out=y0b[:], in_=y0f[:]))

        # ---- q0 (scaled), block-diag columns bd, fused qk -------------------
        bd_sb = wp.tile([128, 2, NH], BF16, tag="bd")
        for m in range(2):
            qps = qp.tile([128, NH], F32, tag="small")
            for c in range(2):
                chain("pe", nc.tensor.matmul(
                    qps[:, 0:1], lhsT=wq_sb[:, c, bass.ts(m, 128)],
                    rhs=y0b[:, c, :], start=(c == 0), stop=(c == 1)))
            q0c = wp.tile([128, 1], BF16, tag="q0c")
            chain("act", nc.scalar.mul(q0c[:], qps[:, 0:1], SCALE))
            chain("dve", nc.vector.tensor_tensor(out=bd_sb[:, m, :],
                                                 in0=q0c[:].to_broadcast([128, NH]),
                                                 in1=maskc_sb[:, m, :],
                                                 op=mybir.AluOpType.mult))
        qk_sb = wp.tile([128, 2, NH], BF16, tag="qk")
        for m in range(2):
            qkps = qp.tile([128, NH], F32, tag="small")
            for c in range(2):
                chain("pe", nc.tensor.matmul(
                    qkps[:], lhsT=wk_sb[:, c, bass.ts(m, 128)],
                    rhs=bd_sb[:, c, :], start=(c == 0), stop=(c == 1)))
            chain("dve", nc.vector.tensor_copy(out=qk_sb[:, m, :], in_=qkps[:]))

        # ---- rotated qk' for the pe-scores of all 16 tiles ------------------
        # qk'[:,0] = C*qk0 + S*qk1 ; qk'[:,1] = C*qk1 - S*qk0
        qksw = wp.tile([128, 2, NH], BF16, tag="qksw")
        chain("dve", nc.vector.tensor_copy(out=qksw[:], in_=qk_sb[:, ::-1, :]))
        qkp = wp.tile([128, 2, NT, NH], BF16, tag="qkp")
        qtmp = wp.tile([128, 2, NT, NH], BF16, tag="qtmp")
        chain("dve", nc.vector.tensor_tensor(
            out=qkp[:], in0=cs[:, 0],
            in1=qk_sb[:].unsqueeze(2).broadcast_to([128, 2, NT, NH]),
            op=mybir.AluOpType.mult))
        chain("dve", nc.vector.tensor_tensor(
            out=qtmp[:], in0=cs[:, 1],
            in1=qksw[:].unsqueeze(2).broadcast_to([128, 2, NT, NH]),
            op=mybir.AluOpType.mult))
        qkp_add = chain("dve", nc.vector.tensor_tensor(
            out=qkp[:], in0=qkp[:], in1=qtmp[:], op=mybir.AluOpType.add))

        # ---- PSUM state -----------------------------------------------------
        # S/aT column layout: col = n*NT + k  (n-major)
        sps = sp.tile([128, NT + 1, NH], F32)     # scores + per-(k,n) den row
        utps = op.tile([128, 2, 24], F32)         # UT | OF | den

        aT = bigp.tile([128, NT, NH], BF16, tag="at", name="at")
        embT = bigp.tile([128, NT, 2, 128], BF16, tag="embt", name="embt")

        # pe-scores for each group: S[:, :, 4g:4g+4] = pe0h^T @ qk' slice
        def spe_mms():
            for g in range(NG):
                for c in range(2):
                    chain("pe", nc.tensor.matmul(
                        sps[:, bass.ts(g, 4), :],
                        lhsT=pe0h[:, c, :],
                        rhs=qkp[:, c, bass.ts(g, 4), :],
                        start=(c == 0), stop=False,
                        skip_group_check=True))

        # ---- per-group pipeline: transpose -> copy -> emb-scores -> exp ----
        def group_a(g):
            tt = ttp.tile([128, 4, 2, 128], BF16, tag="tt")
            for j in range(4):
                k = 4 * g + j
                for c in range(2):
                    chain("pe", nc.tensor.transpose(tt[:, j, c, :],
                                                    ep[:, k, bass.ts(c, 128)],
                                                    ident_sb))
            if g < 3:
                dst = embT[:, bass.ts(g, 4), :, :]
                chain("dve", nc.vector.tensor_copy(out=dst, in_=tt[:]))
            else:
                # split halves across DVE and Act so the copy finishes sooner
                chain("dve", nc.vector.tensor_copy(
                    out=embT[:, 4 * g:4 * g + 2, :, :], in_=tt[:, 0:2, :, :]))
                chain("act", nc.scalar.copy(
                    embT[:, 4 * g + 2:4 * g + 4, :, :], tt[:, 2:4, :, :]))

        def group_be(g):
            for j in range(4):
                k = 4 * g + j
                for c in range(2):
                    chain("pe", nc.tensor.matmul(sps[:, k, :],
                                                 lhsT=embT[:, k, c, :],
                                                 rhs=qk_sb[:, c, :],
                                                 start=False,
                                                 stop=(c == 1 and j == 3),
                                                 skip_group_check=True))
            chain("act", nc.scalar.activation(
                out=aT[:, bass.ts(g, 4), :],
                in_=sps[:, bass.ts(g, 4), :],
                func=mybir.ActivationFunctionType.Exp))

        def group_bu(g):
            # per-(n,k) partial denominators (col g, rows n*4+kk)
            chain("pe", nc.tensor.matmul(sps[0:32, NT, g:g + 1],
                                         lhsT=aT[:, bass.ts(g, 4), :],
                                         rhs=ones_sb[:],
                                         start=True, stop=True,
                                         skip_group_check=True))
            # U^T accumulation: emb tiles + rotated-pe tiles, same psum
            for j in range(4):
                k = 4 * g + j
                for c in range(2):
                    chain("pe", nc.tensor.matmul(utps[:, c, 0:NH],
                                                 lhsT=ep[:, k, bass.ts(c, 128)],
                                                 rhs=aT[:, k, :],
                                                 start=(k == 0 and c == 0),
                                                 stop=False,
                                                 skip_group_check=True))
                    chain("pe", nc.tensor.matmul(utps[:, c, 0:NH],
                                                 lhsT=pep_sb[:, k,
                                                             bass.ts(c, 128)],
                                                 rhs=aT[:, k, :],
                                                 start=False,
                                                 stop=(k == NT - 1 and c == 1),
                                                 skip_group_check=True))

        group_a(0)
        group_a(1)
        spe_mms()
        group_be(0)
        group_a(2)
        group_be(1)
        group_a(3)
        group_bu(0)
        group_bu(1)
        group_be(2)
        group_be(3)
        group_bu(2)
        group_bu(3)

        # ---- softmax denominator + z chain (high priority) ------------------
        # denominator path runs on Act (free after the exps); the U/cross
        # path stays on DVE; they rejoin at oc.
        with tc.high_priority():
            # U^T psum -> SBUF as soon as the accumulation closes
            utsb = wp.tile([128, 2, NH], BF16, tag="utsb")
            chain("dve", nc.vector.tensor_copy(out=utsb[:],
                                               in_=utps[:, :, 0:NH]))
            dsb = wp.tile([32, NG], BF16, tag="dsb")
            chain("act", nc.scalar.copy(dsb[:], sps[0:32, NT, 0:NG]))
            for g in range(NG):
                chain("pe", nc.tensor.matmul(utps[0:NH, 0, 16:17],
                                             lhsT=nmask_sb,
                                             rhs=dsb[:, g:g + 1],
                                             start=(g == 0),
                                             stop=(g == NG - 1),
                                             skip_group_check=True))
            den_sb = wp.tile([NH, 1], F32, tag="den_sb")
            chain("act", nc.scalar.copy(den_sb[:], utps[0:NH, 0, 16:17]))
            recip = wp.tile([NH, 1], F32, tag="recip")
            chain("dve", nc.vector.reciprocal(recip[:], den_sb[:]))
            rexp_sb = wp.tile([128, 2, 1], F32, tag="rexp")
            for c in range(2):
                rexps = qp.tile([128, NH], F32, tag="small")
                chain("pe", nc.tensor.matmul(rexps[:, 0:1],
                                             lhsT=mt[:, bass.ts(c, 128)],
                                             rhs=recip[:], start=True,
                                             stop=True,
                                             skip_group_check=True))
                chain("act", nc.scalar.copy(rexp_sb[:, c, :], rexps[:, 0:1]))
            # cross products OF[(nk)-chunk m, n] = wv^T @ u
            for m in range(2):
                for c in range(2):
                    chain("pe", nc.tensor.matmul(
                        utps[:, m, 8:16],
                        lhsT=wv_sb[:, c, bass.ts(m, 128)],
                        rhs=utsb[:, c, :],
                        start=(c == 0), stop=(c == 1),
                        skip_group_check=True))
            om = wp.tile([128, 2, NH], F32, tag="om")
            chain("dve", nc.vector.tensor_tensor(out=om[:],
                                                 in0=utps[:, :, 8:16],
                                                 in1=maskc_sb[:],
                                                 op=mybir.AluOpType.mult))
            osel = wp.tile([128, 2, 1], F32, tag="osel")
            chain("dve", nc.vector.tensor_reduce(out=osel[:], in_=om[:],
                                                 axis=mybir.AxisListType.X,
                                                 op=mybir.AluOpType.add))
            oc = wp.tile([128, 2, 1], BF16, tag="oc")
            chain("dve", nc.vector.tensor_tensor(out=oc[:], in0=osel[:],
                                                 in1=rexp_sb[:],
                                                 op=mybir.AluOpType.mult))
            for m in range(2):
                zps = qp.tile([128, NH], F32, tag="small")
                for c in range(2):
                    chain("pe", nc.tensor.matmul(
                        zps[:, 0:1],
                        lhsT=wo_sb[:, c, bass.ts(m, 128)],
                        rhs=oc[:, c, :],
                        start=(c == 0), stop=(c == 1),
                        skip_group_check=True))
                zw.append(chain("dve", nc.vector.scalar_tensor_tensor(
                    out=zcol[:, m:m + 1], in0=y0f[:, m, :], scalar=2.0,
                    in1=zps[:, 0:1], op0=mybir.AluOpType.mult,
                    op1=mybir.AluOpType.add)))

        # ---- fire the exchange ---------------------------------------------
        t1 = nc.gpsimd.trigger_dma(count=1)
        for w in zw:
            add_dep_helper(t1.ins, w.ins, reason="fire r1 after z writes")
        add_dep_helper(t1.ins, d1.ins, reason="r1 descs before trigger")
        t2 = nc.gpsimd.trigger_dma(count=1)
        add_dep_helper(t2.ins, t1.ins, reason="round order")
        add_dep_helper(t2.ins, d2.ins, reason="r2 descs before trigger")
        t3 = nc.gpsimd.trigger_dma(count=1)
        add_dep_helper(t3.ins, t2.ins, reason="round order")
        add_dep_helper(t3.ins, d3.ins, reason="r3 descs before trigger")
        rmap_i = chain("sp", nc.sync.dma_start(rankmap[:], zt[0:1, :, 2]))
        add_dep_helper(rmap_i.ins, t3.ins, reason="rankmap after rounds")

        # ---- vocab-parallel unembed, streamed per wu chunk ------------------
        striggers = []
        first_mms = []
        for half in range(2):
            ups = up.tile([128, 128], F32)
            for ch in range(NCH // 2):
                chg = half * (NCH // 2) + ch
                mm_hi = chain("pe", nc.tensor.matmul(
                    ups[:, 8 * ch:8 * ch + 8],
                    lhsT=wu_hi_sb[:, bass.ts(chg, 128)],
                    rhs=zt[:, :, 0], start=True, stop=False,
                    skip_group_check=True))
                add_dep_helper(mm_hi.ins, t3.ins,
                               reason="unembed after exchange rounds fired")
                first_mms.append(mm_hi)
                chain("pe", nc.tensor.matmul(
                    ups[:, 8 * ch:8 * ch + 8],
                    lhsT=wu_lo_sb[:, bass.ts(chg, 128)],
                    rhs=zt[:, :, 1], start=False, stop=True,
                    skip_group_check=True))
            if half == 0:
                oc_i = chain("dve", nc.vector.tensor_copy(out=osb[:, half, :],
                                                          in_=ups[:]))
            else:
                oc_i = chain("act", nc.scalar.copy(osb[:, half, :], ups[:]))
            st = chain("sp", nc.sync.dma_start(
                out[128 * half:128 * (half + 1), :], osb[:, half, :]))
            striggers.append(st)

    if attach_exchange_wait:
        # cross-core arrival gates; attached post-scheduling (the Tile
        # scheduler's single-core sim cannot satisfy them)
        t2.wait_op(rsem1, 2, "sem-ge", check=False)
        t3.wait_op(rsem2, 2, "sem-ge", check=False)
        rmap_i.wait_op(rsem3, 2, "sem-ge", check=False)
        for mm in first_mms:
            mm.wait_op(rsem1, 2, "sem-ge", check=False)
            mm.wait_op(rsem2, 2, "sem-ge", check=False)
            mm.wait_op(rsem3, 2, "sem-ge", check=False)
    nc.finalize()
    return nc


def _pos_encoding_np():
    pos = np.arange(SEQ, dtype=np.float32)[:, None]
    div = np.exp(np.arange(0, H, 2, dtype=np.float32)
                 * np.float32(-(math.log(10000.0) / H)))
    ang = pos * div[None, :]
    pe = np.zeros((SEQ, H), dtype=np.float32)
    pe[:, 0::2] = np.sin(ang)
    pe[:, 1::2] = np.cos(ang)
    return pe


def _part_chunk(a2d):
    """[256, N] -> [128, 2, N] with [p, c, :] = a2d[c*128 + p]."""
    n = a2d.shape[1]
    return np.ascontiguousarray(a2d.reshape(2, 128, n).transpose(1, 0, 2))


def prepare_in_maps(x, emb_w, wq, wk, wv, wo, wu):
    x = np.asarray(x)
    emb_w = np.asarray(emb_w, dtype=np.float32)
    wq = np.asarray(wq, dtype=np.float32)
    wk = np.asarray(wk, dtype=np.float32)
    wv = np.asarray(wv, dtype=np.float32)
    wo = np.asarray(wo, dtype=np.float32)
    wu = np.asarray(wu, dtype=np.float32)

    tok = np.concatenate(
        [np.full((B, 1), 2, dtype=np.int64), x], axis=1).astype(np.int16)

    emb_host = emb_w[:, PERM].astype(ml_dtypes.bfloat16)

    fp8 = ml_dtypes.float8_e4m3fn
    pep_ = _pos_encoding_np()[:, PERM]                 # [2048, 256]
    div = np.exp(np.arange(0, H, 2, dtype=np.float64)
                 * (-(math.log(10000.0) / H)))
    kk = 128.0 * np.arange(NT, dtype=np.float64)
    Cr = np.cos(kk[:, None] * div[None, :])            # [16, 128]
    Sr = np.sin(kk[:, None] * div[None, :])

    pack8_host = np.zeros((128, 2, P8), dtype=fp8)
    pack8_host[:, :, P_WQ:P_WQ + H] = _part_chunk(
        wq.reshape(H, H).T[PERM]).astype(fp8)
    pack8_host[:, :, P_WK:P_WK + H] = _part_chunk(
        wk.reshape(H, H)[:, PERM]).astype(fp8)
    pack8_host[:, :, P_WV:P_WV + H] = _part_chunk(
        wv.reshape(H, H).T[PERM]).astype(fp8)
    pack8_host[:, :, P_WO:P_WO + H] = _part_chunk(
        wo.T[:, PERM]).astype(fp8)
    # ctst[p, 0, c, k, n] = C[k, p]; [p, 1, 0, k, n] = +S[k,p], [p,1,1] = -S
    ctst_host = np.zeros((128, 2, 2, NT, NH), dtype=fp8)
    ctst_host[:, 0] = np.broadcast_to(
        Cr.T[:, None, :, None], (128, 2, NT, NH)).astype(fp8)
    st = np.stack([Sr.T, -Sr.T], axis=1)               # [128, 2, 16]
    ctst_host[:, 1] = np.broadcast_to(
        st[:, :, :, None], (128, 2, NT, NH)).astype(fp8)
    # pe0 hidden-major: [p=h-in-chunk, c, pos]
    pack8_host[:, :, P_PE0:P_PE0 + 128] = _part_chunk(
        pep_[:128].T).astype(fp8)
    # full pe, pos-major tiles: pep[t, k, h] = pe[128k+t, PERM[h]]
    pep_host = np.ascontiguousarray(
        pep_.reshape(NT, 128, H).transpose(1, 0, 2)).astype(fp8)
    # identity in bf16, byte-packed into two fp8 columns per element
    ident_bf = np.eye(128).astype(ml_dtypes.bfloat16)
    pack8_host[:, 0, P_ID:P_ID + 256] = ident_bf.view(fp8)
    hd_idx = np.arange(H) // HD
    maskc = np.ascontiguousarray(
        (hd_idx.reshape(2, 128)[:, :, None] == np.arange(NH)[None, None, :])
        .astype(fp8).transpose(1, 0, 2))               # [128,2,8]
    pack8_host[:, :, P_MC:P_MC + NH] = maskc
    # nmask[row, n]: row = kk*8 + n  ->  n = row % 8
    rows = np.arange(32)
    nmask = (rows[:, None] % 8 == np.arange(NH)[None, :]).astype(fp8)
    pack8_host[0:32, 0, P_NM:P_NM + NH] = nmask

    maskt_host = (hd_idx[None, :] == np.arange(NH)[:, None]).astype(np.float32)
    e2c = _part_chunk(emb_w[2][PERM].reshape(H, 1))    # [128,2,1]

    wu_pad = np.zeros((VPAD, H), dtype=np.float32)
    wu_pad[:V] = wu
    wu_perm = wu_pad[:, PERM]

    # scatter identity idx tables: [16 ch-partitions, 8] per half, tiled to 128
    # (both halves use idx 0..127; the half offset lives in the out AP)
    scat_idx = np.zeros((128, 16), dtype=np.int16)
    for hh in range(2):
        t = np.arange(128, dtype=np.int16).reshape(8, 16).T   # [16, 8]
        scat_idx[:, 8 * hh:8 * hh + 8] = np.tile(t, (8, 1))

    in_maps = []
    for core in range(NCORES):
        tb = tok[core]
        idx_t = np.zeros((128, 144), dtype=np.int16)
        idx_t[:, 0:128] = np.tile(
            np.ascontiguousarray(tb.reshape(SEQ // 16, 16).T), (8, 1))
        idx_t[:, 128:144] = scat_idx
        packm_host = np.zeros((128, 3), dtype=np.float32)
        packm_host[:, 0:2] = e2c[:, :, 0]
        packm_host[0, 2] = float(core)
        sl = wu_perm[VSLICE * core: VSLICE * (core + 1)]           # [4096, 256]
        in_maps.append({
            "emb": emb_host, "idxs": idx_t,
            "pack8": pack8_host, "ctst": ctst_host, "pep": pep_host,
            "packm": packm_host, "maskt": maskt_host,
            "wu_hi": np.ascontiguousarray(sl[:, 0:128].T).astype(
                ml_dtypes.bfloat16),
            "wu_lo": np.ascontiguousarray(sl[:, 128:256].T).astype(fp8),
        })
    return in_maps


def get_nc():
    if "nc" not in _CACHE:
        _CACHE["nc"] = _build()
    return _CACHE["nc"]


def get_timing_nc():
    """Variant without the cross-core sem waits, for single-core TimelineSim.

    Slightly optimistic: it omits the waits for peers' payload arrival
    (~1-2us of skew on real hardware).
    """
    return _build(attach_exchange_wait=False)


def assemble(results):
    full = np.zeros((B, VPAD), dtype=np.float32)
    for core in range(NCORES):
        ranks = np.asarray(results[core]["rankmap"]).astype(np.int32).ravel()
        o = np.asarray(results[core]["out"]).astype(np.float32)
        o = o.reshape(2, 128, 16, 8)                   # [half, p, ch, b]
        blk = o.transpose(3, 0, 2, 1).reshape(B, VSLICE)
        for slot in range(B):
            full[ranks[slot], VSLICE * core: VSLICE * (core + 1)] = blk[slot]
    return np.ascontiguousarray(full[:, :V])


def kernel(x, emb_w, wq, wk, wv, wo, wu):
    nc = get_nc()
    in_maps = prepare_in_maps(x, emb_w, wq, wk, wv, wo, wu)
    res = bass_utils.run_bass_kernel_spmd(
        nc, in_maps, core_ids=list(range(NCORES)))
    return assemble(res.results)
